# revision 1
# baseline (speedup 1.0000x reference)
"""Trainium2 Bass kernel for nn_BasePolicyNetwork (Dirichlet policy head).

Reference computation:
    state = concat([bias[:,None], weight], 1)          # [N, 513]
    v     = state @ wv.T                               # [N, 20]  (q,k are dead code)
    alpha = softmax(v + prior, axis=1)                 # Dirichlet concentrations
    g     = jax.random.gamma(key(42), alpha)
    out   = g / g.sum(1, keepdims=True)

Device strategy (pure data parallel over N across 8 NeuronCores):
  - Host transposes/packs weight so each 2 MiB block load is one fully
    contiguous DRAM read; each core streams its 16 MiB fp16 shard on
    the sync HWDGE ring (11 large jobs -- job boundaries cost ~0.85us
    each and consumers wait for FULL job completion, so few big jobs
    beat many small ones; splitting the stream across both HWDGE rings
    does NOT help: the rings share a ~400 GB/s per-core HBM budget) and
    computes v_w.T [20, 16384] on the TensorEngine, accumulating the
    512-deep contraction across 8 PSUM banks. fp16 halves the DMA bytes
    (the bottleneck); the ~3e-4 rel error on the concentrations is far
    inside the rejection sampler's measured tolerance.
  - PSUM->SBUF f16 casts alternate DVE / ACT so a single cast chain
    (~860ns per [20,512] tile, only 20 of 128 partitions active) never
    paces PSUM-bank reuse against the warm (2.4 GHz) PE; stores go
    per-block on the scalar ring; the final 4 blocks are 512 rows so
    the un-overlapped tail stays short.
  - The rank-1 bias channel contribution (bias x wv[:,0]) is folded in
    on the host (0.002% of the FLOPs).
  - The Dirichlet sampling tail (softmax + gamma + normalize) must be
    bit-compatible with the reference's jax.random.gamma rejection
    sampler, so it runs through the exact same jax op sequence with
    threefry keys on CPU jax (see comment in kernel()).
"""

import os
import sys

for _p in ("/opt/trn_rl_repo",):
    if _p not in sys.path and os.path.isdir(_p):
        sys.path.insert(0, _p)

import numpy as np

N_TOTAL = 131072
N_CORES = 8
R = N_TOTAL // N_CORES  # 16384 rows per core
K_W = 512               # weight channels on device
C = 20                  # output channels
BIGF = 2048             # rows per state DMA chunk (2 MiB at fp16)
RT = 512                # rows per matmul / psum tile
NBIG = R // BIGF        # 8
SUBT = BIGF // RT       # 4

_MM_DT_NAME = os.environ.get("KERNEL_MM_DTYPE", "float16")

_BLOCKS = [BIGF] * (NBIG - 1) + [RT] * 3 + [256, 128, 128]
assert sum(_BLOCKS) == R

_NP_DT = {
    "float32": np.float32,
    "float32r": np.float32,
    "float16": np.float16,
}

_BUILT = {}


def _build():
    if "nc" in _BUILT:
        return _BUILT["nc"]

    import concourse.mybir as mybir
    import concourse.tile as tile
    from concourse import bacc

    mm_dt = getattr(mybir.dt, _MM_DT_NAME)
    f32 = mybir.dt.float32

    nc = bacc.Bacc("TRN2", target_bir_lowering=False, debug=False,
                   num_devices=N_CORES)

    f16 = mybir.dt.float16
    weightT = nc.dram_tensor("weightT", [K_W * R], mm_dt, kind="ExternalInput")
    wvp = nc.dram_tensor("wvp", [128, 4 * C], mm_dt, kind="ExternalInput")
    vout = nc.dram_tensor("vout", [C, R], f16, kind="ExternalOutput")

    blocks = _BLOCKS

    with tile.TileContext(nc) as tc:
        with (
            tc.tile_pool(name="constp", bufs=1) as constp,
            tc.tile_pool(name="statep", bufs=6) as statep,
            tc.tile_pool(name="outp", bufs=1) as outp,
            tc.tile_pool(name="psump", bufs=8, space="PSUM") as psump,
        ):
            wv_sb = constp.tile([128, 4 * C], mm_dt)
            nc.gpsimd.dma_start(wv_sb[:], wvp[:])

            out_sb = outp.tile([C, R], f16)

            st_flat = weightT.ap()

            # PSUM -> SBUF f16 casts alternate DVE / ACT so a single cast
            # chain never paces PSUM-bank reuse against the warm PE.
            cast_ops = [
                nc.vector.tensor_copy,
                lambda o, i: nc.scalar.copy(o, i),
            ]

            r0 = 0
            off = 0
            ti = 0
            store_r0 = 0
            for bi, blk in enumerate(blocks):
                if blk == BIGF:
                    st_sb = statep.tile([128, 4, BIGF], mm_dt, tag="st")
                else:
                    # exact-shape tail tiles: slicing a 2048-wide buffer
                    # would emit sub-1KiB descriptors (2x latency class)
                    st_sb = statep.tile([128, 4, blk], mm_dt, tag=f"st{bi}",
                                        bufs=1, name=f"st_{bi}")
                src = st_flat[off:off + 128 * 4 * blk].rearrange(
                    "(p c n) -> p c n", p=128, c=4
                )
                nc.sync.dma_start(st_sb[:, :, :blk], src)
                off += 128 * 4 * blk
                for s in range((blk + RT - 1) // RT):
                    w = min(RT, blk - s * RT)
                    rt0 = r0 + s * RT
                    ps = psump.tile([C, RT], f32, tag="ps")
                    for c in range(4):
                        nc.tensor.matmul(
                            ps[:, :w],
                            wv_sb[:, c * C:(c + 1) * C],
                            st_sb[:, c, s * RT:s * RT + w],
                            start=(c == 0),
                            stop=(c == 3),
                        )
                    cast_ops[ti % 2](out_sb[:, rt0:rt0 + w], ps[:, :w])
                    ti += 1
                r0 += blk
                # one store per block, except the final three tiny blocks
                # (256/128/128) share a single store: three back-to-back
                # ~670ns store configs on the scalar sequencer would
                # serialize right at the end of the critical tail chain.
                if bi < len(blocks) - 3 or bi == len(blocks) - 1:
                    nc.scalar.dma_start(
                        vout[:, store_r0:r0], out_sb[:, store_r0:r0]
                    )
                    store_r0 = r0

    nc.compile()
    _BUILT["nc"] = nc
    return nc


def _run_device(weight_packs, wvp: np.ndarray, trace: bool = False):
    from concourse import bass_utils

    nc = _build()
    in_maps = [{"weightT": weight_packs[i], "wvp": wvp} for i in range(N_CORES)]
    res = bass_utils.run_bass_kernel_spmd(
        nc, in_maps, core_ids=list(range(N_CORES)), trace=trace,
    )
    v = np.empty((N_TOTAL, C), np.float32)
    for i in range(N_CORES):
        v[i * R:(i + 1) * R] = res.results[i]["vout"].T.astype(np.float32)
    return v, res


def _pack_inputs(bias, weight, wv):
    np_dt = _NP_DT[_MM_DT_NAME]
    w16 = weight.astype(np_dt)
    n_big = NBIG - 1
    packs = []
    for i in range(N_CORES):
        shard = w16[i * R:(i + 1) * R]
        pack = np.empty(R * K_W, np_dt)
        big = pack[:n_big * BIGF * K_W].reshape(n_big, 128, 4, BIGF)
        big[:] = shard[:n_big * BIGF].reshape(
            n_big, BIGF, 4, 128).transpose(0, 3, 2, 1)
        off = n_big * BIGF * K_W
        r0 = n_big * BIGF
        for blk in _BLOCKS[n_big:]:
            seg = pack[off:off + blk * K_W].reshape(128, 4, blk)
            seg[:] = shard[r0:r0 + blk].reshape(blk, 4, 128).transpose(2, 1, 0)
            off += blk * K_W
            r0 += blk
        packs.append(pack)
    wvp = np.empty((128, 4 * C), np_dt)
    for c in range(4):
        wvp[:, c * C:(c + 1) * C] = wv[:, 1 + c * 128: 1 + (c + 1) * 128].T
    return packs, wvp


def kernel(bias, weight, prior, wq, wk, wv, rel_h, rel_w):
    import jax
    import jax.numpy as jnp

    bias = np.asarray(bias, np.float32)
    weight = np.asarray(weight, np.float32)
    prior = np.asarray(prior, np.float32)
    wv = np.asarray(wv, np.float32)

    weightT, wvp = _pack_inputs(bias, weight, wv)
    v, _ = _run_device(weightT, wvp)

    v = v + bias[:, None] * wv[None, :, 0]

    with jax.default_device(jax.devices("cpu")[0]):
        concen = jnp.asarray(v)
        new_concen = jax.nn.softmax(concen + jnp.asarray(prior), axis=1)
        key = jax.random.key(42, impl="threefry2x32")
        g = jax.random.gamma(key, new_concen)
        out = g / jnp.sum(g, axis=1, keepdims=True)
        return np.asarray(out, np.float32)

